# revision 4
# baseline (speedup 1.0000x reference)
"""DeepseekV2 MLA attention (prefill, causal) on 8 trn2 NeuronCores — v2.

Changes vs v1 (the ReduceScatter/AllGather baseline):

* Collective restructure.  All pre-attention exchange is ONE AllToAll:
  each core computes, for its own 256-token slice, the per-head q
  (all 16 heads, rope applied), k_nope and v (all heads) plus the
  shared k_pe, and AllToAll's them so core d ends with its 2 heads'
  q/k_nope and the shared k_pe/v for ALL tokens.  The output side
  replaces the 16.8 MB ReduceScatter with a 1 MB AllToAll of the
  per-head attention outputs o^T; every core then computes its own
  256-row slice of y = o @ w_o locally (needs the full w_o, which is
  prefetched during the attention phase).  Wire bytes per core drop
  ~10x (31 MB -> ~4 MB).

* bf16 everywhere on-chip (fp32 accumulation in PSUM).  Weights are
  converted to bf16 on the host; activations are written back from
  PSUM as bf16.  Same PE throughput as float32r, but half the DMA,
  half the SBUF, half the collective bytes, and 2x DVE on 16-bit ops.

* x is transposed on the host (free) - no PE transposes.

* Causal masking by binary bf16 mask-multiply on the exp() output
  (cheaper than fp32 mask-add on PSUM; exp cannot overflow since
  |scores| < 8 for this operator family).

Layout: everything on-chip is feature-major [feature | position] so
matmuls chain without transposes; softmax runs on transposed scores
[k | q] with denominators from an all-ones matmul (pre-broadcast over
partitions); no max-subtraction needed.
"""
import sys

sys.path.insert(0, "/opt/trn_rl_repo")

import numpy as np

import concourse.bass as bass
from concourse import bacc
import concourse.mybir as mybir
import concourse.tile as tile
from concourse.bass_utils import run_bass_kernel_spmd

F32 = mybir.dt.float32
BF = mybir.dt.bfloat16
AF = mybir.ActivationFunctionType

B, S, E, H = 1, 2048, 2048, 16
DN, DR, DV, R, QLR = 128, 64, 128, 512, 1536
EPS = 1e-6
NCORES = 8
TOK = S // NCORES         # 256 tokens per core
HPC = H // NCORES         # 2 heads per core
SM_SCALE = (DN + DR) ** -0.5
ROPE_BASE = 10000.0

EC = E // 128             # 16 contraction chunks over E
QRC = QLR // 128          # 12 chunks of q_a features
CRC = R // 128            # 4 chunks of ckv features
NQC = S // 512            # 4 query column chunks
NKT = S // 128            # 16 key tiles
QF = H * (DN + DR)        # 3072 q out-features (pair-major permuted)
KNF = H * DN              # 2048
VF = H * DV               # 2048

# AllToAll block layout (per destination core): rows within a block
QOFF, KNOFF, KPOFF, VOFF = 0, HPC * (DN + DR), HPC * (DN + DR) + HPC * DN, 0
KPOFF = HPC * (DN + DR) + HPC * DN          # 384 + 256 = 640
VOFF = KPOFF + DR                           # 704
BR = VOFF + TOK                             # 960 rows per block


def _rope_tables():
    inv_freq = 1.0 / (ROPE_BASE ** (np.arange(0, DR, 2, dtype=np.float64) / DR))
    ang = np.arange(S, dtype=np.float64)[:, None] * inv_freq[None, :]
    cos = np.concatenate([np.cos(ang), np.cos(ang)], -1).astype(np.float32)  # [S,DR]
    sin = np.concatenate([np.sin(ang), np.sin(ang)], -1).astype(np.float32)
    return cos.T.copy(), sin.T.copy()  # [DR, S] feature-major


def _consts():
    # rot(v)[j] = -v[j+32] for j<32 ; v[j-32] for 32<=j<64, as lhsT[k,m]
    p = np.zeros((64, 64), dtype=np.float32)
    for j in range(32):
        p[j + 32, j] = -1.0
    for j in range(32, 64):
        p[j - 32, j] = 1.0
    prot = np.zeros((128, 128), dtype=np.float32)
    prot[:64, :64] = p
    prot[64:, 64:] = p
    # binary causal masks for scoresT tiles [k 128 | q 512]; m = kt - 4*qc
    ii = np.arange(128)[:, None]
    jj = np.arange(512)[None, :]
    masks = np.stack(
        [np.where(jj - ii - 128 * m >= 0, 1.0, 0.0).astype(np.float32)
         for m in range(4)])
    bf = mybir.dt.np(BF)
    return prot.astype(bf), masks.astype(bf)


def _build(skip_collectives=False):
    nc = bacc.Bacc(None, num_devices=NCORES)

    xT_sl = nc.dram_tensor("xT_sl", [E, TOK], BF, kind="ExternalInput")
    w_qa = nc.dram_tensor("w_qa", [E, QLR], BF, kind="ExternalInput")
    w_kva = nc.dram_tensor("w_kva", [E, R + DR], BF, kind="ExternalInput")
    w_qb = nc.dram_tensor("w_qb", [QLR, QF], BF, kind="ExternalInput")
    w_uk = nc.dram_tensor("w_uk", [R, KNF], BF, kind="ExternalInput")
    w_uv = nc.dram_tensor("w_uv", [R, VF], BF, kind="ExternalInput")
    w_o = nc.dram_tensor("w_o", [VF, E], BF, kind="ExternalInput")
    lnw_q = nc.dram_tensor("lnw_q", [QLR, 1], F32, kind="ExternalInput")
    lnw_kv = nc.dram_tensor("lnw_kv", [R, 1], F32, kind="ExternalInput")
    cos2_sl = nc.dram_tensor("cos2_sl", [128, TOK], F32, kind="ExternalInput")
    sin2_sl = nc.dram_tensor("sin2_sl", [128, TOK], F32, kind="ExternalInput")
    ones_in = nc.dram_tensor("ones_in", [128, 128], BF, kind="ExternalInput")
    y_sl = nc.dram_tensor("y_sl", [TOK, E], F32, kind="ExternalOutput")

    prot_np, masks_np = _consts()
    prot_t = nc.inline_tensor(prot_np, name="prot_c")
    masks_t = nc.inline_tensor(masks_np, name="masks_c")

    a2a_in = nc.dram_tensor("a2a_in", [NCORES * BR, TOK], BF)
    a2a_out = nc.dram_tensor("a2a_out", [NCORES * BR, TOK], BF)
    oa_in = nc.dram_tensor("oa_in", [VF, TOK], BF)
    oa_out = nc.dram_tensor("oa_out", [VF, TOK], BF)

    with tile.TileContext(nc) as tc:
        with tc.tile_pool(name="consts", bufs=1) as cp:
            ones_sb = cp.tile([128, 128], BF)
            nc.sync.dma_start(out=ones_sb, in_=ones_in[:, :])
            prot_sb = cp.tile([128, 128], BF)
            nc.sync.dma_start(out=prot_sb, in_=prot_t[:, :])
            eps_sb = cp.tile([128, 1], F32)
            nc.vector.memset(eps_sb[:], EPS)
            lnwq_sb = cp.tile([128, QRC], F32)
            nc.sync.dma_start(
                out=lnwq_sb, in_=lnw_q.rearrange("(rc p) one -> p rc one", p=128))
            lnwkv_sb = cp.tile([128, CRC], F32)
            nc.sync.dma_start(
                out=lnwkv_sb, in_=lnw_kv.rearrange("(rc p) one -> p rc one", p=128))
            cos2_sb = cp.tile([128, TOK], F32)
            nc.sync.dma_start(out=cos2_sb, in_=cos2_sl[:, :])
            sin2_sb = cp.tile([128, TOK], F32)
            nc.sync.dma_start(out=sin2_sb, in_=sin2_sl[:, :])

            # persistent stage-A outputs
            with tc.tile_pool(name="persistA", bufs=1) as pp:
                qa_n = pp.tile([128, QRC, TOK], BF)     # rmsnormed q_a, feat-major
                ckv_n = pp.tile([128, CRC, TOK], BF)    # rmsnormed ckv
                kpe_out = pp.tile([64, TOK], BF)        # roped k_pe

                # weights used after stage A1; allocated up-front, DMAs issued
                # in need-order behind the stage-A1 loads on the SP queue
                with tc.tile_pool(name="wqbp", bufs=1) as wqbp:
                    wqb_sb = wqbp.tile([128, QRC, QF], BF, tag="wqb")
                    wuk_sb = wqbp.tile([128, CRC, KNF], BF, tag="wuk")
                    wuv_sb = wqbp.tile([128, CRC, VF], BF, tag="wuv")

                    # ------------- stage A1: q_a / ckv / k_pe (own tokens) ----
                    with tc.tile_pool(name="pa", bufs=2) as pa, \
                         tc.tile_pool(name="psA", bufs=2, space="PSUM") as psA:
                        xT = pa.tile([128, EC, TOK], BF, tag="xT", bufs=1)
                        nc.sync.dma_start(
                            out=xT, in_=xT_sl.rearrange("(kc p) s -> p kc s", p=128))
                        wkva_sb = pa.tile([128, EC, R + DR], BF, tag="wkva", bufs=1)
                        nc.sync.dma_start(
                            out=wkva_sb,
                            in_=w_kva.rearrange("(kc p) m -> p kc m", p=128))
                        wqa_sb = pa.tile([128, EC, QLR], BF, tag="wqa", bufs=1)
                        nc.sync.dma_start(
                            out=wqa_sb, in_=w_qa.rearrange("(kc p) m -> p kc m", p=128))
                        nc.sync.dma_start(
                            out=wuk_sb, in_=w_uk.rearrange("(rc p) m -> p rc m", p=128))
                        nc.sync.dma_start(
                            out=wuv_sb, in_=w_uv.rearrange("(rc p) m -> p rc m", p=128))
                        nc.sync.dma_start(
                            out=wqb_sb, in_=w_qb.rearrange("(kc p) m -> p kc m", p=128))

                        def feat_major_block(w_sb, col_off, nchunks, lnw_sb, nfeat,
                                             out_tile):
                            raw = pa.tile([128, nchunks, TOK], BF,
                                          tag=f"raw{nfeat}", bufs=1)
                            ssq = psA.tile([128, TOK], F32, tag=f"ssq{nfeat}", bufs=1)
                            for rc in range(nchunks):
                                pq = psA.tile([128, TOK], F32, tag="pq", bufs=2)
                                for kc in range(EC):
                                    nc.tensor.matmul(
                                        pq[:],
                                        w_sb[:, kc, col_off + rc * 128:
                                             col_off + (rc + 1) * 128],
                                        xT[:, kc, :],
                                        start=(kc == 0), stop=(kc == EC - 1))
                                if rc % 2 == 0:
                                    nc.vector.tensor_copy(raw[:, rc, :], pq[:])
                                else:
                                    nc.scalar.copy(raw[:, rc, :], pq[:])
                                sq = pa.tile([128, TOK], BF, tag="sq", bufs=2)
                                nc.scalar.activation(out=sq, in_=pq[:], func=AF.Square)
                                nc.tensor.matmul(ssq[:], ones_sb[:], sq[:],
                                                 start=(rc == 0),
                                                 stop=(rc == nchunks - 1))
                            rstd = pa.tile([128, TOK], F32, tag=f"rstd{col_off}",
                                           bufs=1)
                            nc.scalar.activation(out=rstd, in_=ssq[:], func=AF.Sqrt,
                                                 scale=1.0 / nfeat, bias=eps_sb[:])
                            nc.vector.reciprocal(rstd[:], rstd[:])
                            for rc in range(nchunks):
                                nc.vector.scalar_tensor_tensor(
                                    out=out_tile[:, rc, :], in0=raw[:, rc, :],
                                    scalar=lnw_sb[:, rc:rc + 1], in1=rstd[:],
                                    op0=mybir.AluOpType.mult,
                                    op1=mybir.AluOpType.mult)

                        feat_major_block(wkva_sb, 0, CRC, lnwkv_sb, R, ckv_n)

                        # k_pe + rope
                        ppe = psA.tile([64, TOK], F32, tag="ppe", bufs=1)
                        for kc in range(EC):
                            nc.tensor.matmul(ppe[:], wkva_sb[:, kc, R:R + DR],
                                             xT[:, kc, :],
                                             start=(kc == 0), stop=(kc == EC - 1))
                        kpe_raw = pa.tile([64, TOK], BF, tag="kpe_raw", bufs=1)
                        nc.scalar.copy(kpe_raw[:], ppe[:])
                        prot_ps = psA.tile([64, TOK], F32, tag="prot_ps", bufs=1)
                        nc.tensor.matmul(prot_ps[:], prot_sb[0:64, 0:64], kpe_raw[:],
                                         start=True, stop=True)
                        t1 = pa.tile([64, TOK], F32, tag="t1", bufs=1)
                        nc.vector.tensor_mul(t1[:], kpe_raw[:], cos2_sb[0:64, :])
                        t2 = pa.tile([64, TOK], F32, tag="t2", bufs=1)
                        nc.vector.tensor_mul(t2[:], prot_ps[:], sin2_sb[0:64, :])
                        nc.vector.tensor_add(kpe_out[:], t1[:], t2[:])
                        # k_pe is replicated to every destination block; ship now
                        for d in range(NCORES):
                            nc.sync.dma_start(
                                out=a2a_in[d * BR + KPOFF:d * BR + KPOFF + DR, :],
                                in_=kpe_out[:])

                        feat_major_block(wqa_sb, 0, QRC, lnwq_sb, QLR, qa_n)

                    # ------------- stage A2: q (all heads) + kn + v ----------
                    # stage_qkn holds the q+kn rows of every destination block
                    # contiguously (per d: 3 q chunks then 2 kn chunks), so one
                    # DMA per destination ships both; staging DMAs are issued
                    # as soon as their chunks are ready.
                    with tc.tile_pool(name="qp", bufs=1) as qp, \
                         tc.tile_pool(name="psQ", bufs=2, space="PSUM") as psQ:
                        stqkn = qp.tile([128, 5 * NCORES, TOK], BF, tag="stqkn")

                        def q_chunk(mc):
                            dst = stqkn[:, 5 * (mc // 3) + mc % 3, :]
                            pq2 = psQ.tile([128, TOK], F32, tag="pq2", bufs=2)
                            for kc in range(QRC):
                                nc.tensor.matmul(
                                    pq2[:], wqb_sb[:, kc, mc * 128:(mc + 1) * 128],
                                    qa_n[:, kc, :],
                                    start=(kc == 0), stop=(kc == QRC - 1))
                            if mc % 3 < 2:
                                if mc % 2 == 0:
                                    nc.vector.tensor_copy(dst, pq2[:])
                                else:
                                    nc.scalar.copy(dst, pq2[:])
                            else:
                                qpe_raw = qp.tile([128, TOK], BF, tag="qpe_raw",
                                                  bufs=2)
                                nc.scalar.copy(qpe_raw[:], pq2[:])
                                rot_ps = psQ.tile([128, TOK], F32, tag="rot_ps",
                                                  bufs=2)
                                nc.tensor.matmul(rot_ps[:], prot_sb[:], qpe_raw[:],
                                                 start=True, stop=True)
                                tq1 = qp.tile([128, TOK], F32, tag="tq1", bufs=2)
                                nc.gpsimd.tensor_mul(tq1[:], qpe_raw[:], cos2_sb[:])
                                tq2 = qp.tile([128, TOK], F32, tag="tq2", bufs=2)
                                nc.vector.tensor_mul(tq2[:], rot_ps[:], sin2_sb[:])
                                nc.vector.tensor_add(dst, tq1[:], tq2[:])

                        def kn_head(h):
                            dst = stqkn[:, 5 * (h // 2) + 3 + h % 2, :]
                            pk = psQ.tile([128, TOK], F32, tag="pk", bufs=2)
                            for rc in range(CRC):
                                nc.tensor.matmul(
                                    pk[:], wuk_sb[:, rc, h * DN:(h + 1) * DN],
                                    ckv_n[:, rc, :],
                                    start=(rc == 0), stop=(rc == CRC - 1))
                            if h % 2 == 0:
                                nc.vector.tensor_copy(dst, pk[:])
                            else:
                                nc.scalar.copy(dst, pk[:])

                        for d in range(NCORES):
                            for mc in (3 * d, 3 * d + 1, 3 * d + 2):
                                q_chunk(mc)
                            kn_head(2 * d)
                            kn_head(2 * d + 1)
                            nc.sync.dma_start(
                                out=a2a_in[d * BR:d * BR + KPOFF, :].rearrange(
                                    "(cc p) s -> p cc s", p=128),
                                in_=stqkn[:, 5 * d:5 * d + 5, :])

                        vA = qp.tile([128, TOK // 128, VF], BF, tag="vA")
                        for fc in range(VF // 512):
                            for tch in range(TOK // 128):
                                pv = psQ.tile([128, 512], F32, tag="pv", bufs=2)
                                for rc in range(CRC):
                                    nc.tensor.matmul(
                                        pv[:], ckv_n[:, rc, tch * 128:(tch + 1) * 128],
                                        wuv_sb[:, rc, fc * 512:(fc + 1) * 512],
                                        start=(rc == 0), stop=(rc == CRC - 1))
                                if tch % 2 == 0:
                                    nc.vector.tensor_copy(
                                        vA[:, tch, fc * 512:(fc + 1) * 512], pv[:])
                                else:
                                    nc.scalar.copy(
                                        vA[:, tch, fc * 512:(fc + 1) * 512], pv[:])
                            for d in (2 * fc, 2 * fc + 1):
                                nc.scalar.dma_start(
                                    out=a2a_in[d * BR + VOFF:d * BR + VOFF + TOK, :]
                                    .rearrange("(cc p) f -> p cc f", p=128),
                                    in_=vA[:, :, d * TOK:(d + 1) * TOK])

            if skip_collectives:
                nc.gpsimd.dma_start(out=a2a_out[:, :], in_=a2a_in[:, :])
            else:
                nc.gpsimd.collective_compute(
                    "AllToAll", mybir.AluOpType.bypass,
                    replica_groups=[list(range(NCORES))],
                    ins=[a2a_in[:, :].opt()], outs=[a2a_out[:, :].opt()])

            av = a2a_out.rearrange("(c r) s -> r c s", c=NCORES)  # [BR, 8, TOK]

            # ---------------- stage C: attention + oT ----------------
            with tc.tile_pool(name="ab", bufs=1) as ab, \
                 tc.tile_pool(name="psC", bufs=1, space="PSUM") as psC:
                wo_sb = ab.tile([128, EC, E], BF, tag="wo")
                nc.scalar.dma_start(
                    out=wo_sb, in_=w_o.rearrange("(fc p) e -> p fc e", p=128))

                qnT = [ab.tile([128, S], BF, tag=f"qnT{h}", name=f"qnT{h}")
                       for h in range(HPC)]
                nc.sync.dma_start(out=qnT[0], in_=av[0:128, :, :])
                nc.sync.dma_start(out=qnT[1], in_=av[128:256, :, :])
                qpeT = ab.tile([128, S], BF, tag="qpeT")
                nc.sync.dma_start(out=qpeT, in_=av[256:384, :, :])
                qpe_h1 = ab.tile([64, S], BF, tag="qpe_h1")
                nc.sync.dma_start(out=qpe_h1, in_=av[320:384, :, :])
                knT = [ab.tile([128, S], BF, tag=f"knT{h}", name=f"knT{h}")
                       for h in range(HPC)]
                nc.sync.dma_start(out=knT[0], in_=av[KNOFF:KNOFF + 128, :, :])
                nc.sync.dma_start(out=knT[1], in_=av[KNOFF + 128:KNOFF + 256, :, :])
                kpeT = ab.tile([64, S], BF, tag="kpeT")
                nc.sync.dma_start(out=kpeT, in_=av[KPOFF:KPOFF + DR, :, :])
                # v_sb[p, tch, j, f]: token = (2*j + tch)*128 + p
                v_sb = ab.tile([128, 2, NCORES, HPC * DV], BF, tag="v_sb")
                nc.scalar.dma_start(out=v_sb[:, 0, :, :],
                                    in_=av[VOFF:VOFF + 128, :, :])
                nc.scalar.dma_start(out=v_sb[:, 1, :, :],
                                    in_=av[VOFF + 128:VOFF + 256, :, :])
                mask_sb = ab.tile([128, 4, 512], BF, tag="mask")
                nc.sync.dma_start(out=mask_sb,
                                  in_=masks_t.rearrange("m p f -> p m f"))

                oT = ab.tile([128, HPC, S], BF, tag="oT")
                for qc in range(NQC):
                    cs = slice(qc * 512, (qc + 1) * 512)
                    nkt = 4 * qc + 4
                    for h in range(HPC):
                        po = psC.tile([128, 512], F32, tag="po", bufs=2)
                        pdn = psC.tile([128, 512], F32, tag="pdn", bufs=2)
                        for kt in range(nkt):
                            ks = slice(kt * 128, (kt + 1) * 128)
                            ps = psC.tile([128, 512], F32, tag="ps", bufs=2)
                            nc.tensor.matmul(ps[:], knT[h][:, ks], qnT[h][:, cs],
                                             start=True, stop=False)
                            qpe_rhs = qpeT[0:64, cs] if h == 0 else qpe_h1[:, cs]
                            nc.tensor.matmul(ps[:], kpeT[:, ks], qpe_rhs,
                                             start=False, stop=True)
                            et = ab.tile([128, 512], BF, tag="et", bufs=3)
                            nc.scalar.activation(out=et, in_=ps[:], func=AF.Exp,
                                                 scale=SM_SCALE)
                            m = kt - 4 * qc
                            if m >= 0:
                                nc.gpsimd.tensor_mul(et[:], et[:], mask_sb[:, m, :])
                            nc.tensor.matmul(po[:],
                                             v_sb[:, kt % 2, kt // 2,
                                                  h * DV:(h + 1) * DV],
                                             et[:], start=(kt == 0),
                                             stop=(kt == nkt - 1))
                            nc.tensor.matmul(pdn[:], ones_sb[:], et[:],
                                             start=(kt == 0), stop=(kt == nkt - 1))
                        rec = ab.tile([128, 512], F32, tag="rec", bufs=2)
                        nc.vector.reciprocal(rec[:], pdn[:])
                        nc.vector.tensor_mul(oT[:, h, cs], po[:], rec[:])
                    # ship this q-chunk's o blocks
                    for d in (2 * qc, 2 * qc + 1):
                        nc.scalar.dma_start(
                            out=oa_in[d * TOK:(d + 1) * TOK, :].rearrange(
                                "(hc p) t -> p hc t", p=128),
                            in_=oT[:, :, d * TOK:(d + 1) * TOK])

                if skip_collectives:
                    nc.gpsimd.dma_start(out=oa_out[:, :], in_=oa_in[:, :])
                else:
                    nc.gpsimd.collective_compute(
                        "AllToAll", mybir.AluOpType.bypass,
                        replica_groups=[list(range(NCORES))],
                        ins=[oa_in[:, :].opt()], outs=[oa_out[:, :].opt()])

                # ------------- stage D: y_sl = o_sl @ w_o -------------
                oTa = ab.tile([128, EC, TOK], BF, tag="oTa")
                nc.sync.dma_start(
                    out=oTa, in_=oa_out.rearrange("(fc p) t -> p fc t", p=128))
                for qt in range(TOK // 128):
                    for ecn in range(E // 512):
                        py = psC.tile([128, 512], F32, tag="py", bufs=2)
                        for fc in range(EC):
                            nc.tensor.matmul(
                                py[:], oTa[:, fc, qt * 128:(qt + 1) * 128],
                                wo_sb[:, fc, ecn * 512:(ecn + 1) * 512],
                                start=(fc == 0), stop=(fc == EC - 1))
                        y_sb = ab.tile([128, 512], F32, tag="y_sb", bufs=3)
                        if ecn % 2 == 0:
                            nc.vector.tensor_copy(y_sb[:], py[:])
                        else:
                            nc.scalar.copy(y_sb[:], py[:])
                        nc.sync.dma_start(
                            out=y_sl[qt * 128:(qt + 1) * 128,
                                     ecn * 512:(ecn + 1) * 512],
                            in_=y_sb[:])
    nc.finalize()
    return nc


_NC_CACHE = None


def _get_nc():
    global _NC_CACHE
    if _NC_CACHE is None:
        _NC_CACHE = _build()
    return _NC_CACHE


def _make_in_maps(x, w_q_a, q_a_ln_w, w_q_b, w_kv_a, kv_a_ln_w, w_kv_b, w_o):
    bf = mybir.dt.np(BF)
    x = np.asarray(x, dtype=np.float32)
    w_qa_b = np.ascontiguousarray(np.asarray(w_q_a, np.float32)).astype(bf)
    w_kva_b = np.ascontiguousarray(np.asarray(w_kv_a, np.float32)).astype(bf)
    wqb = np.asarray(w_q_b, np.float32).reshape(QLR, H, DN + DR)
    parts = []
    for d in range(NCORES):
        h0, h1 = 2 * d, 2 * d + 1
        parts += [wqb[:, h0, :DN], wqb[:, h1, :DN], wqb[:, h0, DN:], wqb[:, h1, DN:]]
    w_qb_perm = np.ascontiguousarray(np.concatenate(parts, axis=1)).astype(bf)
    wkv = np.asarray(w_kv_b, np.float32).reshape(R, H, DN + DV)
    w_uk_b = np.ascontiguousarray(wkv[..., :DN].reshape(R, H * DN)).astype(bf)
    w_uv_b = np.ascontiguousarray(wkv[..., DN:].reshape(R, H * DV)).astype(bf)
    w_o_b = np.ascontiguousarray(np.asarray(w_o, np.float32)).astype(bf)

    cosT, sinT = _rope_tables()
    in_maps = []
    for c in range(NCORES):
        sl = slice(c * TOK, (c + 1) * TOK)
        cos2 = np.ascontiguousarray(np.concatenate([cosT[:, sl], cosT[:, sl]], 0))
        sin2 = np.ascontiguousarray(np.concatenate([sinT[:, sl], sinT[:, sl]], 0))
        in_maps.append({
            "xT_sl": np.ascontiguousarray(x[0, sl, :].T).astype(bf),
            "w_qa": w_qa_b,
            "w_kva": w_kva_b,
            "w_qb": w_qb_perm,
            "w_uk": w_uk_b,
            "w_uv": w_uv_b,
            "w_o": w_o_b,
            "lnw_q": np.ascontiguousarray(
                np.asarray(q_a_ln_w, np.float32).reshape(QLR, 1)),
            "lnw_kv": np.ascontiguousarray(
                np.asarray(kv_a_ln_w, np.float32).reshape(R, 1)),
            "cos2_sl": cos2,
            "sin2_sl": sin2,
            "ones_in": np.ones((128, 128), np.float32).astype(bf),
        })
    return in_maps


def kernel(**inputs):
    in_maps = _make_in_maps(**inputs)
    nc = _get_nc()
    # The axon terminal occasionally reports NRT_EXEC_UNIT_UNRECOVERABLE on the
    # first load after a prior session died; a retry recovers it.
    last_exc = None
    for _ in range(3):
        try:
            res = run_bass_kernel_spmd(nc, in_maps, core_ids=list(range(NCORES)))
            break
        except Exception as e:  # noqa: BLE001
            last_exc = e
    else:
        raise last_exc
    y = np.concatenate([res.results[c]["y_sl"] for c in range(NCORES)], axis=0)
    return y.reshape(B, S, E).astype(np.float32)


if __name__ == "__main__":
    nc = _build()
    print("built ok")
